# revision 1
# baseline (speedup 1.0000x reference)
"""Trainium2 Bass kernel for a 6-layer post-BatchNorm transformer encoder.

Reference model:
  x = emb[seq] + pes                                  # [B,S,D] = [4,512,1024]
  6x: x = BN(x + attn(x)); x = BN(x + ffn(x))
  BN = per-channel batch stats over (B,S), eps=1e-3.

Sharding: tensor-parallel across 8 NeuronCores. Each core owns H/8=2 heads
(QKV out / Wo in slices) and DF/8=512 FFN hidden units. After Wo and after
W2 an fp32 AllReduce combines partial [D, T] outputs; the residual x/8 is
folded into each partial via an extra (1/8)*I matmul so the AllReduce
directly yields x + sublayer(x). bo/b2 biases cancel inside BN and are
dropped. BatchNorm is computed redundantly on every core, keeping the
program SPMD-uniform (no rank-dependent addressing anywhere).

Activation layout: transposed. x^T lives in SBUF as [128 part, 8 dtile,
2048 tok] so natural-layout weights serve directly as matmul lhsT
(stationary) and activations as rhs (moving); no per-layer activation
transposes. Attention per (batch, head): scores^T = K_h @ Q_h^T,
E = exp(scale*scores^T) (softmax max-subtraction skipped; scores are O(1)),
U^T = V_h^T @ E^T with column sums from a ones-row matmul, normalized by a
PE-broadcast reciprocal row. Matmuls run as float32r (full-rate fp32 PE
mode; plain fp32 is 4x slower).
"""

import os

import numpy as np

import concourse.bass as bass
import concourse.mybir as mybir
import concourse.tile as tile
from concourse import bacc
from concourse.bass import ts
from concourse.masks import make_identity

# ---------------------------------------------------------------- dims
V, D, L, H, B, S = 32000, 1024, 6, 16, 4, 512
HD = D // H            # 64
DF = 4 * D             # 4096
EPS = 1e-3
NC = 8                 # cores
T = B * S              # 2048 tokens
P = 128                # partitions
DT = D // P            # 8 d-tiles
TT = T // P            # 16 token tiles
CH = 512               # token chunk (matmul N)
NCH = T // CH          # 4 chunks
HPC = H // NC          # heads per core = 2
DSH = HPC * HD         # qkv out shard = 128
FSH = DF // NC         # ffn hidden shard = 512
FMT = FSH // P         # ffn1 m-tiles = 4
KL = FSH // P          # ffn2 k-tiles = 4

f32 = mybir.dt.float32
f16 = mybir.dt.float16
f32r = mybir.dt.float32r
i16 = mybir.dt.int16
AF = mybir.ActivationFunctionType
ALU = mybir.AluOpType

REPLICAS = [list(range(NC))]

N_LAYERS = int(os.environ.get("TRN_KERNEL_LAYERS", str(L)))
DEBUG_TAPS = os.environ.get("TRN_KERNEL_DEBUG", "0") == "1"

GATHER_QUEUES = int(os.environ.get("TRN_GATHER_QUEUES", "1"))


def _r(ap):
    """view an fp32 AP as float32r for full-rate PE matmul"""
    return ap.bitcast(f32r)


def build_module(n_layers=None):
    if n_layers is None:
        n_layers = N_LAYERS
    nc = bacc.Bacc("TRN2", target_bir_lowering=False, debug=False,
                   num_devices=NC)

    dt_ = nc.dram_tensor
    io = {
        "emb": dt_("emb", [V, D], f32, kind="ExternalInput").ap(),
        "idx": dt_("idx", [16, T // 16], i16, kind="ExternalInput").ap(),
        "pesT": dt_("pesT", [D, S], f32, kind="ExternalInput").ap(),
        "wq": dt_("wq", [L, D, DSH], f32, kind="ExternalInput").ap(),
        "wk": dt_("wk", [L, D, DSH], f32, kind="ExternalInput").ap(),
        "wv": dt_("wv", [L, D, DSH], f32, kind="ExternalInput").ap(),
        "wo": dt_("wo", [L, DSH, D], f32, kind="ExternalInput").ap(),
        "w1": dt_("w1", [L, D, FSH], f32, kind="ExternalInput").ap(),
        "w2": dt_("w2", [L, FSH, D], f32, kind="ExternalInput").ap(),
        "bq": dt_("bq", [L, DSH], f32, kind="ExternalInput").ap(),
        "bk": dt_("bk", [L, DSH], f32, kind="ExternalInput").ap(),
        "bv": dt_("bv", [L, DSH], f32, kind="ExternalInput").ap(),
        "b1": dt_("b1", [L, FSH], f32, kind="ExternalInput").ap(),
        "g1": dt_("g1", [L, D], f32, kind="ExternalInput").ap(),
        "be1": dt_("be1", [L, D], f32, kind="ExternalInput").ap(),
        "g2": dt_("g2", [L, D], f32, kind="ExternalInput").ap(),
        "be2": dt_("be2", [L, D], f32, kind="ExternalInput").ap(),
        "out": dt_("out", [D, T], f32, kind="ExternalOutput").ap(),
    }
    if DEBUG_TAPS:
        for nm, shp in [("dbg_x", [D, T]), ("dbg_q", [P, T]), ("dbg_k", [P, T]),
                        ("dbg_v", [P, TT * DSH]), ("dbg_attn", [P, T]),
                        ("dbg_y1", [D, T]), ("dbg_x2", [D, T])]:
            io[nm] = dt_(nm, shp, f32, kind="ExternalOutput").ap()

    with tile.TileContext(nc) as tc:
        _build(tc, n_layers, io)
    nc.compile()
    return nc


def _build(tc, n_layers, io):
    from contextlib import ExitStack
    nc = tc.nc
    att_scale = 1.0 / np.sqrt(HD)

    # ------------------------------------------------ pools
    st = ExitStack()
    persist = st.enter_context(tc.tile_pool(name="persist", bufs=1))
    wpool = st.enter_context(tc.tile_pool(name="wpool", bufs=1))   # W1/W2
    wqkv = st.enter_context(tc.tile_pool(name="wqkv", bufs=1))     # Wq/Wk/Wv/Wo
    small = st.enter_context(tc.tile_pool(name="small", bufs=2))   # biases/stats
    tok8k = st.enter_context(tc.tile_pool(name="tok8k", bufs=2))   # [128, T]
    e512 = st.enter_context(tc.tile_pool(name="e512", bufs=6))     # [128, CH]
    htp = st.enter_context(tc.tile_pool(name="htp", bufs=2))       # [128,FMT,CH]
    ps = st.enter_context(tc.tile_pool(name="ps", bufs=5, space="PSUM"))
    pst = st.enter_context(tc.tile_pool(name="pst", bufs=2, space="PSUM"))
    drin = st.enter_context(tc.tile_pool(name="drin", bufs=2, space="DRAM"))
    drout = st.enter_context(tc.tile_pool(name="drout", bufs=2, space="DRAM"))

    # ------------------------------------------------ persistent tiles
    xbuf = persist.tile([P, DT, T], f32, name="xbuf")      # x / x2 (fp32)
    qT = persist.tile([P, T], f32, name="qT")              # Q^T shard
    kT = persist.tile([P, T], f32, name="kT")              # K^T shard
    vsb = persist.tile([P, TT, 2 * (HD + 1)], f32, name="vsb")  # [V|1|V|1]
    ident = persist.tile([P, P], f32, name="ident")
    eye8 = persist.tile([P, P], f32, name="eye8")
    onesP64 = persist.tile([P, 64], f32, name="onesP64")
    attnTA = persist.tile([HD, T], f32, name="attnTA")     # head-0 attn^T
    attnTB = persist.tile([HD, T], f32, name="attnTB")     # head-1 attn^T
    idxs = persist.tile([P, T // 16], i16, name="idxs")

    make_identity(nc, ident[:])
    nc.scalar.mul(_r(eye8[:]), ident[:], 1.0 / NC)         # (1/8) * I
    nc.vector.memset(onesP64[:], 1.0)
    nc.scalar.activation(_r(vsb[:, :, HD:HD + 1]), ident[:, 0:TT].unsqueeze(-1),
                         AF.Identity, bias=1.0, scale=0.0)
    nc.scalar.activation(_r(vsb[:, :, 2 * HD + 1:]), ident[:, 0:TT].unsqueeze(-1),
                         AF.Identity, bias=1.0, scale=0.0)
    # indices wrapped in 16 partitions, replicated into all 8 Q7-core stripes
    for r_ in range(P // 16):
        nc.sync.dma_start(idxs[16 * r_:16 * (r_ + 1), :], io["idx"])

    # ---------------------------------------- embedding: x^T = (emb[seq])^T + pes^T
    pes_lo = htp.tile([P, FMT, CH], f32, tag="ht", name="pes_lo")
    pes_hi = htp.tile([P, FMT, CH], f32, tag="ht", name="pes_hi")
    pes_r = io["pesT"].rearrange("(k p) s -> p k s", p=P)
    nc.sync.dma_start(pes_lo[:], pes_r[:, 0:4, :])
    nc.sync.dma_start(pes_hi[:], pes_r[:, 4:8, :])

    for half in range(TT // 2):  # gather 2 token-tiles (256 rows) at a time
        gtile = tok8k.tile([P, 2, D], f32, tag="tok", name=f"gt{half}")
        nc.gpsimd.dma_gather(
            out_ap=gtile[:],
            in_ap=io["emb"],
            idxs_ap=idxs[:, half * 16:(half + 1) * 16],
            num_idxs=2 * P,
            num_idxs_reg=2 * P,
            elem_size=D,
            queue_num=half % GATHER_QUEUES,
        )
        for j in range(2):
            t = half * 2 + j            # token tile index
            pos_t = t % (S // P)        # position tile within the batch
            for k in range(DT):
                ptile = pst.tile([P, P], f32, tag="tp", name=f"tp{t}_{k}")
                nc.tensor.transpose(ptile[:], gtile[:, j, ts(k, P)], ident[:])
                pes_src = pes_lo if k < 4 else pes_hi
                nc.vector.tensor_tensor(
                    out=_r(xbuf[:, k, ts(t, P)]),
                    in0=ptile[:],
                    in1=pes_src[:, k % 4, ts(pos_t, P)],
                    op=ALU.add,
                )

    if DEBUG_TAPS:
        nc.sync.dma_start(io["dbg_x"].rearrange("(k p) t -> p k t", p=P), xbuf[:])

    # ---------------------------------------- batchnorm (redundant, full-D)
    def batchnorm(lbl, arout_t, g_sb, be_sb):
        ysum = small.tile([P, DT], f32, tag="ysum", name=f"ysum{lbl}")
        sqp = small.tile([P, DT, NCH], f32, tag="sqp", name=f"sqp{lbl}")
        for k in range(DT):
            yt = tok8k.tile([P, T], f32, tag="tok", name=f"yt{lbl}_{k}")
            nc.sync.dma_start(yt[:], arout_t[ts(k, P), :])
            nc.vector.reduce_sum(out=ysum[:, k:k + 1], in_=yt[:],
                                 axis=mybir.AxisListType.X)
            for c in range(NCH):
                scr = e512.tile([P, CH], f32, tag="e", name=f"sq{lbl}_{k}_{c}")
                nc.scalar.activation(scr[:], yt[:, ts(c, CH)], AF.Square,
                                     accum_out=sqp[:, k, c:c + 1])
        sq = small.tile([P, DT], f32, tag="sq", name=f"sq{lbl}")
        nc.vector.reduce_sum(out=sq[:], in_=sqp[:], axis=mybir.AxisListType.X)
        mean = small.tile([P, DT], f32, tag="mean", name=f"mean{lbl}")
        nc.vector.tensor_scalar_mul(mean[:], ysum[:], 1.0 / T)
        msq = small.tile([P, DT], f32, tag="msq", name=f"msq{lbl}")
        nc.vector.tensor_tensor(out=msq[:], in0=mean[:], in1=mean[:], op=ALU.mult)
        veps = small.tile([P, DT], f32, tag="veps", name=f"veps{lbl}")
        # veps = sq/T - mean^2 + EPS
        nc.vector.scalar_tensor_tensor(out=veps[:], in0=sq[:], scalar=1.0 / T,
                                       in1=msq[:], op0=ALU.mult, op1=ALU.subtract)
        nc.vector.tensor_scalar_add(veps[:], veps[:], EPS)
        rec = small.tile([P, DT], f32, tag="rec", name=f"rec{lbl}")
        nc.vector.reciprocal(rec[:], veps[:])
        rstd = small.tile([P, DT], f32, tag="rstd", name=f"rstd{lbl}")
        nc.scalar.sqrt(rstd[:], rec[:])
        sc = small.tile([P, DT], f32, tag="sc", name=f"sc{lbl}")
        nc.vector.tensor_tensor(out=sc[:], in0=g_sb[:], in1=rstd[:], op=ALU.mult)
        sh = small.tile([P, DT], f32, tag="sh", name=f"sh{lbl}")
        nc.vector.tensor_tensor(out=sh[:], in0=mean[:], in1=sc[:], op=ALU.mult)
        nc.vector.tensor_tensor(out=sh[:], in0=be_sb[:], in1=sh[:], op=ALU.subtract)
        for k in range(DT):
            yt = tok8k.tile([P, T], f32, tag="tok", name=f"ya{lbl}_{k}")
            nc.sync.dma_start(yt[:], arout_t[ts(k, P), :])
            nc.scalar.activation(_r(xbuf[:, k, :]), yt[:], AF.Identity,
                                 bias=sh[:, k:k + 1], scale=sc[:, k:k + 1])

    # ---------------------------------------- layers
    for l in range(n_layers):
        # ---- layer weights/params to SBUF
        wq_sb = wqkv.tile([P, DT, DSH], f32, tag="wq", name=f"wq{l}")
        wk_sb = wqkv.tile([P, DT, DSH], f32, tag="wk", name=f"wk{l}")
        wv_sb = wqkv.tile([P, DT, DSH], f32, tag="wv", name=f"wv{l}")
        wo_sbA = wqkv.tile([HD, D], f32, tag="woA", name=f"woA{l}")
        wo_sbB = wqkv.tile([HD, D], f32, tag="woB", name=f"woB{l}")
        w1_sb = wpool.tile([P, DT, FSH], f32, tag="w1", name=f"w1{l}")
        w2_sb = wpool.tile([P, KL, D], f32, tag="w2", name=f"w2{l}")
        nc.sync.dma_start(_r(wq_sb[:]), _r(io["wq"][l].rearrange("(k p) m -> p k m", p=P)))
        nc.sync.dma_start(_r(wk_sb[:]), _r(io["wk"][l].rearrange("(k p) m -> p k m", p=P)))
        nc.sync.dma_start(_r(wv_sb[:]), _r(io["wv"][l].rearrange("(k p) m -> p k m", p=P)))
        nc.sync.dma_start(_r(wo_sbA[:]), _r(io["wo"][l][0:HD, :]))
        nc.sync.dma_start(_r(wo_sbB[:]), _r(io["wo"][l][HD:2 * HD, :]))
        nc.sync.dma_start(_r(w1_sb[:]), _r(io["w1"][l].rearrange("(k p) m -> p k m", p=P)))
        nc.sync.dma_start(_r(w2_sb[:]), _r(io["w2"][l].rearrange("(k p) m -> p k m", p=P)))

        bq_sb = small.tile([P, 1], f32, tag="bq", name=f"bq{l}")
        bk_sb = small.tile([P, 1], f32, tag="bk", name=f"bk{l}")
        bv_sb = small.tile([P, 1], f32, tag="bv", name=f"bv{l}")
        b1_sb = small.tile([P, FMT], f32, tag="b1", name=f"b1{l}")
        nc.sync.dma_start(bq_sb[:], io["bq"][l].rearrange("(p o) -> p o", o=1))
        nc.sync.dma_start(bk_sb[:], io["bk"][l].rearrange("(p o) -> p o", o=1))
        nc.sync.dma_start(bv_sb[:], io["bv"][l].rearrange("(p o) -> p o", o=1))
        nc.sync.dma_start(b1_sb[:], io["b1"][l].rearrange("(m p) -> p m", p=P))

        g1_sb = small.tile([P, DT], f32, tag="g1", name=f"g1{l}")
        be1_sb = small.tile([P, DT], f32, tag="be1", name=f"be1{l}")
        g2_sb = small.tile([P, DT], f32, tag="g2", name=f"g2{l}")
        be2_sb = small.tile([P, DT], f32, tag="be2", name=f"be2{l}")
        nc.sync.dma_start(g1_sb[:], io["g1"][l].rearrange("(k p) -> p k", p=P))
        nc.sync.dma_start(be1_sb[:], io["be1"][l].rearrange("(k p) -> p k", p=P))
        nc.sync.dma_start(g2_sb[:], io["g2"][l].rearrange("(k p) -> p k", p=P))
        nc.sync.dma_start(be2_sb[:], io["be2"][l].rearrange("(k p) -> p k", p=P))

        # ---- QKV projections (shard): Q^T/K^T/V^T = W_shard^T @ x^T
        vT = tok8k.tile([P, T], f32, tag="tok", name=f"vT{l}")
        for c in range(NCH):
            psq = ps.tile([P, CH], f32, tag="mm", name=f"psq{l}_{c}")
            psk = ps.tile([P, CH], f32, tag="mm", name=f"psk{l}_{c}")
            psv = ps.tile([P, CH], f32, tag="mm", name=f"psv{l}_{c}")
            for k in range(DT):
                fl, ll = (k == 0), (k == DT - 1)
                rhs = _r(xbuf[:, k, ts(c, CH)])
                nc.tensor.matmul(psq[:], _r(wq_sb[:, k, :]), rhs, start=fl, stop=ll)
                nc.tensor.matmul(psk[:], _r(wk_sb[:, k, :]), rhs, start=fl, stop=ll)
                nc.tensor.matmul(psv[:], _r(wv_sb[:, k, :]), rhs, start=fl, stop=ll)
            nc.scalar.activation(_r(qT[:, ts(c, CH)]), psq[:], AF.Identity, bias=bq_sb[:])
            nc.scalar.activation(_r(kT[:, ts(c, CH)]), psk[:], AF.Identity, bias=bk_sb[:])
            nc.scalar.activation(vT[:, ts(c, CH)], psv[:], AF.Identity, bias=bv_sb[:])

        # ---- V^T -> V (token-partition layout) via PE transposes
        for t in range(TT):
            ptile = pst.tile([P, P], f32, tag="tp", name=f"vt{l}_{t}")
            nc.tensor.transpose(ptile[:], vT[:, ts(t, P)], ident[:])
            nc.vector.tensor_copy(
                _r(vsb[:, t, :].rearrange("p (h x) -> p h x", h=2)[:, :, 0:HD]),
                ptile[:].rearrange("p (h x) -> p h x", h=2))

        # ---- attention: per head all-f32r at PSUM base 0; softmax sums
        # fused into the U matmul via the ones-column appended to V.
        for b in range(B):
            for h, attnT_h in enumerate([attnTA, attnTB]):
                hp = h * HD
                vof = h * (HD + 1)
                ets = []
                for sk in range(B):
                    pss = ps.tile([P, CH], f32, tag="mm",
                                  name=f"pss{l}_{b}_{h}_{sk}")
                    nc.tensor.matmul(
                        pss[:],
                        _r(kT[hp:hp + HD, b * CH + sk * P:b * CH + (sk + 1) * P]),
                        _r(qT[hp:hp + HD, ts(b, CH)]),
                        start=True, stop=True)
                    et = e512.tile([P, CH], f32, tag="e",
                                   name=f"et{l}_{b}_{h}_{sk}")
                    nc.scalar.activation(_r(et[:]), pss[:], AF.Exp, scale=att_scale)
                    ets.append(et)
                psu = ps.tile([P, CH], f32, tag="mm", name=f"psu{l}_{b}_{h}")
                for sk in range(B):
                    nc.tensor.matmul(psu[0:HD + 1, :],
                                     _r(vsb[:, b * 4 + sk, vof:vof + HD + 1]),
                                     _r(ets[sk][:]),
                                     start=(sk == 0), stop=(sk == B - 1))
                rsb = e512.tile([P, CH], f32, tag="e", name=f"rsb{l}_{b}_{h}")
                nc.vector.reciprocal(rsb[HD:HD + 1, :], psu[HD:HD + 1, :])
                psr = ps.tile([P, CH], f32, tag="mm", name=f"psr{l}_{b}_{h}")
                nc.tensor.matmul(psr[0:HD, :], onesP64[HD:HD + 1, :],
                                 rsb[HD:HD + 1, :], start=True, stop=True)
                usb = e512.tile([P, CH], f32, tag="e", name=f"usb{l}_{b}_{h}")
                nc.scalar.copy(usb[0:HD, :], psu[0:HD, :])
                nc.vector.tensor_tensor(out=_r(attnT_h[:, ts(b, CH)]),
                                        in0=usb[0:HD, :],
                                        in1=psr[0:HD, :], op=ALU.mult)

        if DEBUG_TAPS and l == 0:
            nc.sync.dma_start(io["dbg_q"], qT[:])
            nc.sync.dma_start(io["dbg_k"], kT[:])
            nc.sync.dma_start(io["dbg_v"], vsb[:].rearrange("p a b -> p (a b)"))
            nc.sync.dma_start(io["dbg_attn"], attnTA[:].rearrange("p t -> p t"))

        # ---- Wo partial + residual/8 -> AllReduce
        arin1 = drin.tile([D, T], f32, tag="ari", name=f"ari1_{l}")
        arout1 = drout.tile([D, T], f32, tag="aro", addr_space="Shared",
                            name=f"aro1_{l}")
        for m in range(DT):
            for c in range(NCH):
                ps2 = ps.tile([P, CH], f32, tag="mm", name=f"pso{l}_{m}_{c}")
                nc.tensor.matmul(ps2[:], _r(wo_sbA[:, ts(m, P)]),
                                 _r(attnTA[:, ts(c, CH)]), start=True, stop=False)
                nc.tensor.matmul(ps2[:], _r(wo_sbB[:, ts(m, P)]),
                                 _r(attnTB[:, ts(c, CH)]), start=False, stop=False)
                nc.tensor.matmul(ps2[:], _r(eye8[:]), _r(xbuf[:, m, ts(c, CH)]),
                                 start=False, stop=True)
                osb = e512.tile([P, CH], f32, tag="e", name=f"osb{l}_{m}_{c}")
                nc.vector.tensor_copy(osb[:], ps2[:])
                nc.sync.dma_start(arin1[ts(m, P), ts(c, CH)], osb[:])
        nc.gpsimd.collective_compute(
            "AllReduce", ALU.add, replica_groups=REPLICAS,
            ins=[arin1.opt()], outs=[arout1.opt()])

        if DEBUG_TAPS and l == 0:
            nc.sync.dma_start(io["dbg_y1"], arout1)

        # ---- BN1 -> x2 into xbuf
        batchnorm(f"a{l}", arout1, g1_sb, be1_sb)
        if DEBUG_TAPS and l == 0:
            nc.sync.dma_start(io["dbg_x2"].rearrange("(k p) t -> p k t", p=P),
                              xbuf[:])

        # ---- FFN (chunk-major so h^T is chunk-resident) + residual/8 -> AR
        arin2 = drin.tile([D, T], f32, tag="ari", name=f"ari2_{l}")
        arout2 = drout.tile([D, T], f32, tag="aro", addr_space="Shared",
                            name=f"aro2_{l}")
        for c in range(NCH):
            ht = htp.tile([P, FMT, CH], f32, tag="ht", name=f"ht{l}_{c}")
            for m in range(FMT):
                ps1 = ps.tile([P, CH], f32, tag="mm", name=f"ps1{l}_{c}_{m}")
                for k in range(DT):
                    nc.tensor.matmul(ps1[:], _r(w1_sb[:, k, ts(m, P)]),
                                     _r(xbuf[:, k, ts(c, CH)]),
                                     start=(k == 0), stop=(k == DT - 1))
                nc.scalar.activation(_r(ht[:, m, :]), ps1[:], AF.Relu,
                                     bias=b1_sb[:, m:m + 1])
            for m in range(DT):
                ps2 = ps.tile([P, CH], f32, tag="mm", name=f"ps2{l}_{c}_{m}")
                for k in range(KL):
                    nc.tensor.matmul(ps2[:], _r(w2_sb[:, k, ts(m, P)]),
                                     _r(ht[:, k, :]), start=(k == 0), stop=False)
                nc.tensor.matmul(ps2[:], _r(eye8[:]), _r(xbuf[:, m, ts(c, CH)]),
                                 start=False, stop=True)
                osb = e512.tile([P, CH], f32, tag="e", name=f"fsb{l}_{c}_{m}")
                nc.vector.tensor_copy(osb[:], ps2[:])
                nc.sync.dma_start(arin2[ts(m, P), ts(c, CH)], osb[:])
        nc.gpsimd.collective_compute(
            "AllReduce", ALU.add, replica_groups=REPLICAS,
            ins=[arin2.opt()], outs=[arout2.opt()])

        # ---- BN2 -> x(l+1) into xbuf
        batchnorm(f"f{l}", arout2, g2_sb, be2_sb)

    # ---------------------------------------- output x^T -> [D, T]
    nc.sync.dma_start(io["out"].rearrange("(k p) t -> p k t", p=P), xbuf[:])
    st.close()


# ================================================================ host side

def make_in_maps(inputs):
    f = lambda a: np.ascontiguousarray(np.asarray(a), dtype=np.float32)
    seq = np.asarray(inputs["sequence"]).reshape(-1).astype(np.int16)
    idx = np.ascontiguousarray(seq.reshape(T // 16, 16).T)     # [16, T//16]
    emb = f(inputs["emb"])
    pesT = np.ascontiguousarray(f(inputs["pes"]).T)            # [D, S]
    Wq, Wk, Wv = f(inputs["Wq"]), f(inputs["Wk"]), f(inputs["Wv"])
    Wo, W1, W2 = f(inputs["Wo"]), f(inputs["W1"]), f(inputs["W2"])
    bq, bk, bv = f(inputs["bq"]), f(inputs["bk"]), f(inputs["bv"])
    b1 = f(inputs["b1"])
    g1, be1 = f(inputs["g1"]), f(inputs["be1"])
    g2, be2 = f(inputs["g2"]), f(inputs["be2"])

    in_maps = []
    for c in range(NC):
        ds_ = slice(c * DSH, (c + 1) * DSH)
        fs_ = slice(c * FSH, (c + 1) * FSH)
        in_maps.append({
            "emb": emb,
            "idx": idx,
            "pesT": pesT,
            "wq": np.ascontiguousarray(Wq[:, :, ds_]),
            "wk": np.ascontiguousarray(Wk[:, :, ds_]),
            "wv": np.ascontiguousarray(Wv[:, :, ds_]),
            "wo": np.ascontiguousarray(Wo[:, ds_, :]),
            "w1": np.ascontiguousarray(W1[:, :, fs_]),
            "w2": np.ascontiguousarray(W2[:, fs_, :]),
            "bq": np.ascontiguousarray(bq[:, ds_]),
            "bk": np.ascontiguousarray(bk[:, ds_]),
            "bv": np.ascontiguousarray(bv[:, ds_]),
            "b1": np.ascontiguousarray(b1[:, fs_]),
            "g1": g1, "be1": be1, "g2": g2, "be2": be2,
        })
    return in_maps


_CACHE = {}


def _get_module():
    if "nc" not in _CACHE:
        _CACHE["nc"] = build_module()
    return _CACHE["nc"]


def kernel(**inputs):
    from concourse import bass_utils
    nc = _get_module()
    in_maps = make_in_maps(inputs)
    res = bass_utils.run_bass_kernel_spmd(nc, in_maps, list(range(NC)))
    o = np.asarray(res.results[0]["out"])                  # [D, T]
    return np.ascontiguousarray(o.T).reshape(B, S, D).astype(np.float32)



# revision 5
# speedup vs baseline: 1.8766x; 1.8766x over previous
"""Trainium2 Bass kernel for a 6-layer post-BatchNorm transformer encoder.

Reference model:
  x = emb[seq] + pes                                  # [B,S,D] = [4,512,1024]
  6x: x = BN(x + attn(x)); x = BN(x + ffn(x))
  BN = per-channel batch stats over (B,S), eps=1e-3.

Sharding: dp=4 x tp=2 mesh over 8 NeuronCores. Core c owns sample b=c//2
(512 tokens) and tensor-parallel half t=c%2 (8 heads of QKV/Wo, 2048 of the
4096 FFN hidden units). Per sublayer, the two cores of a pair AllReduce their
partial [D,512] sublayer output (bf16, 1MB) - 8x less collective traffic than
8-way tensor parallelism. The residual is added locally in fp-space after the
AllReduce. BatchNorm needs batch statistics over all 4 samples: each core
computes per-channel sum/sumsq over its sample and an 8KB 8-core AllReduce
combines them (each sample counted twice -> divide by 2T).

Activation layout: transposed, bf16. x^T lives in SBUF as [128 part, 8 dtile,
512 tok]; weights (bf16, host-converted) serve as matmul lhsT directly.
Attention per head: scores^T = K_h @ Q_h^T, E = exp(scale*scores^T),
U^T = V_h^T @ E^T with softmax denominators from a ones-column appended to V,
normalized via a PE-broadcast reciprocal row. PSUM accumulation is fp32;
biases/BN params stay fp32.

Host side shards inputs per core and reassembles the 4 samples from cores
0,2,4,6 - no final gather collective.
"""

import os

import numpy as np

import concourse.bass as bass
import concourse.mybir as mybir
import concourse.tile as tile
from concourse import bacc
from concourse.bass import ts
from concourse.masks import make_identity

# ---------------------------------------------------------------- dims
V, D, L, H, B, S = 32000, 1024, 6, 16, 4, 512
HD = D // H            # 64
DF = 4 * D             # 4096
EPS = 1e-3
NC = 8                 # cores
P = 128                # partitions
T = B * S              # 2048 tokens total
SL = S                 # tokens per core (one sample)
DT = D // P            # 8 d-tiles
TP = 2                 # tensor-parallel width
DPW = NC // TP         # data-parallel width (4 samples)
DSH = D // TP          # qkv out shard = 512
QT = DSH // P          # 4 q-tiles
HPC = H // TP          # heads per core = 8
FSH = DF // TP         # ffn hidden shard = 2048
FMT = FSH // P         # ffn1 m-tiles = 16
KVT = SL // P          # kv token tiles = 4

f32 = mybir.dt.float32
bf16 = mybir.dt.bfloat16
f32r = mybir.dt.float32r
i16 = mybir.dt.int16
AF = mybir.ActivationFunctionType
ALU = mybir.AluOpType

PAIRS = [[0, 1], [2, 3], [4, 5], [6, 7]]
ALL8 = [list(range(NC))]

N_LAYERS = int(os.environ.get("TRN_KERNEL_LAYERS", str(L)))


def build_module(n_layers=None):
    if n_layers is None:
        n_layers = N_LAYERS
    nc = bacc.Bacc("TRN2", target_bir_lowering=False, debug=False,
                   num_devices=NC)

    dt_ = nc.dram_tensor
    io = {
        "emb": dt_("emb", [V, D], f32, kind="ExternalInput").ap(),
        "idx": dt_("idx", [16, SL // 16], i16, kind="ExternalInput").ap(),
        "pesT": dt_("pesT", [D, SL], bf16, kind="ExternalInput").ap(),
        "wq": dt_("wq", [L, D, DSH], bf16, kind="ExternalInput").ap(),
        "wk": dt_("wk", [L, D, DSH], bf16, kind="ExternalInput").ap(),
        "wv": dt_("wv", [L, D, DSH], bf16, kind="ExternalInput").ap(),
        "wo": dt_("wo", [L, DSH, D], bf16, kind="ExternalInput").ap(),
        "w1": dt_("w1", [L, D, FSH], bf16, kind="ExternalInput").ap(),
        "w2": dt_("w2", [L, FSH, D], bf16, kind="ExternalInput").ap(),
        "bq": dt_("bq", [L, DSH], f32, kind="ExternalInput").ap(),
        "bk": dt_("bk", [L, DSH], f32, kind="ExternalInput").ap(),
        "bv": dt_("bv", [L, DSH], f32, kind="ExternalInput").ap(),
        "b1": dt_("b1", [L, FSH], f32, kind="ExternalInput").ap(),
        "g1": dt_("g1", [L, D], f32, kind="ExternalInput").ap(),
        "be1": dt_("be1", [L, D], f32, kind="ExternalInput").ap(),
        "g2": dt_("g2", [L, D], f32, kind="ExternalInput").ap(),
        "be2": dt_("be2", [L, D], f32, kind="ExternalInput").ap(),
        "out": dt_("out", [D, SL], bf16, kind="ExternalOutput").ap(),
    }

    with tile.TileContext(nc) as tc:
        _build(tc, n_layers, io)
    nc.compile()
    return nc


def _build(tc, n_layers, io):
    from contextlib import ExitStack
    nc = tc.nc
    att_scale = 1.0 / np.sqrt(HD)

    # ------------------------------------------------ pools
    st = ExitStack()
    persist = st.enter_context(tc.tile_pool(name="persist", bufs=1))
    wqkv = st.enter_context(tc.tile_pool(name="wqkv", bufs=1))
    wff = st.enter_context(tc.tile_pool(name="wff", bufs=1))
    small = st.enter_context(tc.tile_pool(name="small", bufs=2))
    ybuf = st.enter_context(tc.tile_pool(name="ybuf", bufs=1))   # AR readback
    epool = st.enter_context(tc.tile_pool(name="epool", bufs=6))  # [128,512] bf16
    efpool = st.enter_context(tc.tile_pool(name="efpool", bufs=4))  # [128,512] f32
    hpool = st.enter_context(tc.tile_pool(name="hpool", bufs=1))  # ffn hidden
    ps = st.enter_context(tc.tile_pool(name="ps", bufs=5, space="PSUM"))
    pst = st.enter_context(tc.tile_pool(name="pst", bufs=1, space="PSUM"))
    drin = st.enter_context(tc.tile_pool(name="drin", bufs=2, space="DRAM"))
    drout = st.enter_context(tc.tile_pool(name="drout", bufs=2, space="DRAM"))
    drst = st.enter_context(tc.tile_pool(name="drst", bufs=2, space="DRAM"))

    # ------------------------------------------------ persistent tiles
    xbf = persist.tile([P, DT, SL], bf16, name="xbf")      # x^T (bf16)
    qT = persist.tile([P, QT, SL], bf16, name="qT")
    kT = persist.tile([P, QT, SL], bf16, name="kT")
    vT = persist.tile([P, QT, SL], bf16, name="vT")
    vsb = persist.tile([P, KVT, HPC * (HD + 1)], bf16, name="vsb")
    attnT = persist.tile([P, QT, SL], bf16, name="attnT")
    ident = persist.tile([P, P], f32, name="ident")
    identb = persist.tile([P, P], bf16, name="identb")
    ones1 = persist.tile([P, HD], f32, name="ones1")
    idxs = persist.tile([P, SL // 16], i16, name="idxs")

    make_identity(nc, ident[:])
    nc.vector.tensor_copy(identb[:], ident[:])
    nc.vector.memset(ones1[:], 1.0)
    # ones columns of vsb (col HD of each head block), set once
    for h in range(HPC):
        nc.scalar.activation(vsb[:, :, h * (HD + 1) + HD:h * (HD + 1) + HD + 1],
                             ident[:, 0:KVT].unsqueeze(-1),
                             AF.Identity, bias=1.0, scale=0.0)
    for r_ in range(P // 16):
        nc.sync.dma_start(idxs[16 * r_:16 * (r_ + 1), :], io["idx"])

    # ---------------------------------------- embedding: x^T = (emb[seq])^T + pes^T
    pes_sb = ybuf.tile([P, DT, SL], bf16, tag="yt", name="pes_sb")
    nc.sync.dma_start(pes_sb[:], io["pesT"].rearrange("(k p) s -> p k s", p=P))
    for half in range(KVT // 2):  # gather 256 tokens at a time
        gtile = ybuf.tile([P, 2, D], f32, tag="gt", name=f"gt{half}")
        nc.gpsimd.dma_gather(
            out_ap=gtile[:],
            in_ap=io["emb"],
            idxs_ap=idxs[:, half * 16:(half + 1) * 16],
            num_idxs=2 * P,
            num_idxs_reg=2 * P,
            elem_size=D,
            queue_num=0,
        )
        for j in range(2):
            t = half * 2 + j            # token tile index (= position tile)
            for k in range(DT):
                ptile = pst.tile([P, P], f32, tag="tp", name=f"tp{t}_{k}")
                nc.tensor.transpose(ptile[:], gtile[:, j, ts(k, P)], ident[:])
                nc.vector.tensor_tensor(
                    out=xbf[:, k, ts(t, P)],
                    in0=ptile[:],
                    in1=pes_sb[:, k, ts(t, P)],
                    op=ALU.add,
                )

    # ---------------------------------------- batchnorm
    def batchnorm(lbl, arout_t, g_sb, be_sb):
        # read back pair-reduced sublayer output, residual-add into xbf
        yt = ybuf.tile([P, DT, SL], bf16, tag="yt", name=f"yt{lbl}")
        nc.sync.dma_start(yt[:], arout_t.rearrange("(k p) t -> p k t", p=P))
        nc.vector.tensor_tensor(out=xbf[:], in0=xbf[:], in1=yt[:], op=ALU.add)
        # local stats over my sample's 512 tokens
        stats = small.tile([P, 2 * DT], f32, tag="st", name=f"st{lbl}")
        nc.vector.reduce_sum(out=stats[:, 0:DT].unsqueeze(-1), in_=xbf[:],
                             axis=mybir.AxisListType.X)
        for k in range(DT):
            scr = epool.tile([P, SL], bf16, tag="e", name=f"sq{lbl}_{k}")
            nc.scalar.activation(scr[:], xbf[:, k, :], AF.Square,
                                 accum_out=stats[:, DT + k:DT + k + 1])
        sin = drst.tile([P, 2 * DT], f32, tag="si", name=f"si{lbl}")
        sout = drst.tile([P, 2 * DT], f32, tag="so", addr_space="Shared",
                         name=f"so{lbl}")
        nc.sync.dma_start(sin, stats[:])
        nc.gpsimd.collective_compute(
            "AllReduce", ALU.add, replica_groups=ALL8,
            ins=[sin.opt()], outs=[sout.opt()])
        gstats = small.tile([P, 2 * DT], f32, tag="gs", name=f"gs{lbl}")
        nc.sync.dma_start(gstats[:], sout)
        # finalize: mean/var over 2T (each sample contributed twice)
        mean = small.tile([P, DT], f32, tag="mean", name=f"mean{lbl}")
        nc.vector.tensor_scalar_mul(mean[:], gstats[:, 0:DT], 1.0 / (2 * T))
        msq = small.tile([P, DT], f32, tag="msq", name=f"msq{lbl}")
        nc.vector.tensor_tensor(out=msq[:], in0=mean[:], in1=mean[:], op=ALU.mult)
        veps = small.tile([P, DT], f32, tag="veps", name=f"veps{lbl}")
        nc.vector.scalar_tensor_tensor(out=veps[:], in0=gstats[:, DT:2 * DT],
                                       scalar=1.0 / (2 * T),
                                       in1=msq[:], op0=ALU.mult, op1=ALU.subtract)
        nc.vector.tensor_scalar_add(veps[:], veps[:], EPS)
        rec = small.tile([P, DT], f32, tag="rec", name=f"rec{lbl}")
        nc.vector.reciprocal(rec[:], veps[:])
        rstd = small.tile([P, DT], f32, tag="rstd", name=f"rstd{lbl}")
        nc.scalar.sqrt(rstd[:], rec[:])
        sc = small.tile([P, DT], f32, tag="sc", name=f"sc{lbl}")
        nc.vector.tensor_tensor(out=sc[:], in0=g_sb[:], in1=rstd[:], op=ALU.mult)
        sh = small.tile([P, DT], f32, tag="sh", name=f"sh{lbl}")
        nc.vector.tensor_tensor(out=sh[:], in0=mean[:], in1=sc[:], op=ALU.mult)
        nc.vector.tensor_tensor(out=sh[:], in0=be_sb[:], in1=sh[:], op=ALU.subtract)
        for k in range(DT):
            nc.scalar.activation(xbf[:, k, :], xbf[:, k, :], AF.Identity,
                                 bias=sh[:, k:k + 1], scale=sc[:, k:k + 1])

    # ---------------------------------------- layers
    for l in range(n_layers):
        # ---- layer weights/params to SBUF (bf16)
        wq_sb = wqkv.tile([P, DT, DSH], bf16, tag="wq", name=f"wq{l}")
        wk_sb = wqkv.tile([P, DT, DSH], bf16, tag="wk", name=f"wk{l}")
        wv_sb = wqkv.tile([P, DT, DSH], bf16, tag="wv", name=f"wv{l}")
        wo_sb = wqkv.tile([P, QT, D], bf16, tag="wo", name=f"wo{l}")
        w1_sb = wff.tile([P, DT, FSH], bf16, tag="w1", name=f"w1{l}")
        w2_sb = wff.tile([P, FMT, D], bf16, tag="w2", name=f"w2{l}")
        nc.sync.dma_start(wq_sb[:], io["wq"][l].rearrange("(k p) m -> p k m", p=P))
        nc.sync.dma_start(wk_sb[:], io["wk"][l].rearrange("(k p) m -> p k m", p=P))
        nc.sync.dma_start(wv_sb[:], io["wv"][l].rearrange("(k p) m -> p k m", p=P))
        nc.sync.dma_start(wo_sb[:], io["wo"][l].rearrange("(k p) m -> p k m", p=P))
        nc.sync.dma_start(w1_sb[:], io["w1"][l].rearrange("(k p) m -> p k m", p=P))
        nc.sync.dma_start(w2_sb[:], io["w2"][l].rearrange("(k p) m -> p k m", p=P))

        bq_sb = small.tile([P, QT], f32, tag="bq", name=f"bq{l}")
        bk_sb = small.tile([P, QT], f32, tag="bk", name=f"bk{l}")
        bv_sb = small.tile([P, QT], f32, tag="bv", name=f"bv{l}")
        b1_sb = small.tile([P, FMT], f32, tag="b1", name=f"b1{l}")
        nc.sync.dma_start(bq_sb[:], io["bq"][l].rearrange("(m p) -> p m", p=P))
        nc.sync.dma_start(bk_sb[:], io["bk"][l].rearrange("(m p) -> p m", p=P))
        nc.sync.dma_start(bv_sb[:], io["bv"][l].rearrange("(m p) -> p m", p=P))
        nc.sync.dma_start(b1_sb[:], io["b1"][l].rearrange("(m p) -> p m", p=P))

        g1_sb = small.tile([P, DT], f32, tag="g1", name=f"g1{l}")
        be1_sb = small.tile([P, DT], f32, tag="be1", name=f"be1{l}")
        g2_sb = small.tile([P, DT], f32, tag="g2", name=f"g2{l}")
        be2_sb = small.tile([P, DT], f32, tag="be2", name=f"be2{l}")
        nc.sync.dma_start(g1_sb[:], io["g1"][l].rearrange("(k p) -> p k", p=P))
        nc.sync.dma_start(be1_sb[:], io["be1"][l].rearrange("(k p) -> p k", p=P))
        nc.sync.dma_start(g2_sb[:], io["g2"][l].rearrange("(k p) -> p k", p=P))
        nc.sync.dma_start(be2_sb[:], io["be2"][l].rearrange("(k p) -> p k", p=P))

        # ---- QKV projections (shard): [P, QT, SL] = W_shard^T @ x^T
        for m in range(QT):
            for src, dst, b_sb in ((wq_sb, qT, bq_sb), (wk_sb, kT, bk_sb),
                                   (wv_sb, vT, bv_sb)):
                psq = ps.tile([P, SL], f32, tag="mm", name=f"ps{l}_{m}_{id(dst)}")
                for k in range(DT):
                    nc.tensor.matmul(psq[:], src[:, k, ts(m, P)], xbf[:, k, :],
                                     start=(k == 0), stop=(k == DT - 1))
                nc.scalar.activation(dst[:, m, :], psq[:], AF.Identity,
                                     bias=b_sb[:, m:m + 1])

        # ---- V^T -> V (token-partition layout), 2 heads per chan-tile
        for m in range(QT):
            for tt in range(KVT):
                ptile = pst.tile([P, P], bf16, tag="tpb", name=f"vt{l}_{m}_{tt}")
                nc.tensor.transpose(ptile[:], vT[:, m, ts(tt, P)], identb[:])
                nc.vector.tensor_copy(
                    vsb[:, tt, :].rearrange("p (h x) -> p h x", x=HD + 1)
                    [:, 2 * m:2 * m + 2, 0:HD],
                    ptile[:].rearrange("p (h x) -> p h x", h=2))

        # ---- attention per head
        for h in range(HPC):
            qt_, prow = h // 2, (h % 2) * HD
            vof = h * (HD + 1)
            ets = []
            for kvt in range(KVT):
                pss = ps.tile([P, SL], f32, tag="mm", name=f"pss{l}_{h}_{kvt}")
                nc.tensor.matmul(
                    pss[:],
                    kT[prow:prow + HD, qt_, ts(kvt, P)],
                    qT[prow:prow + HD, qt_, :],
                    start=True, stop=True)
                et = epool.tile([P, SL], bf16, tag="e", name=f"et{l}_{h}_{kvt}")
                nc.scalar.activation(et[:], pss[:], AF.Exp, scale=att_scale)
                ets.append(et)
            psu = ps.tile([P, SL], f32, tag="mm", name=f"psu{l}_{h}")
            for kvt in range(KVT):
                nc.tensor.matmul(psu[0:HD + 1, :],
                                 vsb[:, kvt, vof:vof + HD + 1],
                                 ets[kvt][:],
                                 start=(kvt == 0), stop=(kvt == KVT - 1))
            rsb = efpool.tile([P, SL], f32, tag="ef", name=f"rsb{l}_{h}")
            nc.vector.reciprocal(rsb[HD:HD + 1, :], psu[HD:HD + 1, :])
            psr = ps.tile([P, SL], f32, tag="mm", name=f"psr{l}_{h}")
            nc.tensor.matmul(psr[0:HD, :], ones1[HD:HD + 1, :],
                             rsb[HD:HD + 1, :], start=True, stop=True)
            usb = efpool.tile([P, SL], f32, tag="ef", name=f"usb{l}_{h}")
            nc.scalar.copy(usb[0:HD, :], psu[0:HD, :])
            nc.vector.tensor_tensor(out=attnT[prow:prow + HD, qt_, :],
                                    in0=usb[0:HD, :],
                                    in1=psr[0:HD, :], op=ALU.mult)

        # ---- Wo partial -> pair AllReduce (bf16)
        arin1 = drin.tile([D, SL], bf16, tag="ari", name=f"ari1_{l}")
        arout1 = drout.tile([D, SL], bf16, tag="aro", name=f"aro1_{l}")
        for m in range(DT):
            ps2 = ps.tile([P, SL], f32, tag="mm", name=f"pso{l}_{m}")
            for kt in range(QT):
                nc.tensor.matmul(ps2[:], wo_sb[:, kt, ts(m, P)],
                                 attnT[:, kt, :],
                                 start=(kt == 0), stop=(kt == QT - 1))
            osb = epool.tile([P, SL], bf16, tag="e", name=f"osb{l}_{m}")
            nc.vector.tensor_copy(osb[:], ps2[:])
            nc.sync.dma_start(arin1[ts(m, P), :], osb[:])
        nc.gpsimd.collective_compute(
            "AllReduce", ALU.add, replica_groups=PAIRS,
            ins=[arin1.opt()], outs=[arout1.opt()])

        # ---- BN1 (updates xbf in place)
        batchnorm(f"a{l}", arout1, g1_sb, be1_sb)

        # ---- FFN
        ht = hpool.tile([P, FMT, SL], bf16, tag="ht", name=f"ht{l}")
        for m in range(FMT):
            ps1 = ps.tile([P, SL], f32, tag="mm", name=f"ps1{l}_{m}")
            for k in range(DT):
                nc.tensor.matmul(ps1[:], w1_sb[:, k, ts(m, P)], xbf[:, k, :],
                                 start=(k == 0), stop=(k == DT - 1))
            nc.scalar.activation(ht[:, m, :], ps1[:], AF.Relu,
                                 bias=b1_sb[:, m:m + 1])
        arin2 = drin.tile([D, SL], bf16, tag="ari", name=f"ari2_{l}")
        arout2 = drout.tile([D, SL], bf16, tag="aro", name=f"aro2_{l}")
        for m in range(DT):
            ps2 = ps.tile([P, SL], f32, tag="mm", name=f"ps2{l}_{m}")
            for kt in range(FMT):
                nc.tensor.matmul(ps2[:], w2_sb[:, kt, ts(m, P)], ht[:, kt, :],
                                 start=(kt == 0), stop=(kt == FMT - 1))
            osb = epool.tile([P, SL], bf16, tag="e", name=f"fsb{l}_{m}")
            nc.vector.tensor_copy(osb[:], ps2[:])
            nc.sync.dma_start(arin2[ts(m, P), :], osb[:])
        nc.gpsimd.collective_compute(
            "AllReduce", ALU.add, replica_groups=PAIRS,
            ins=[arin2.opt()], outs=[arout2.opt()])

        # ---- BN2
        batchnorm(f"f{l}", arout2, g2_sb, be2_sb)

    # ---------------------------------------- output x^T -> [D, SL] (bf16)
    nc.sync.dma_start(io["out"].rearrange("(k p) t -> p k t", p=P), xbf[:])
    st.close()


# ================================================================ host side

def _bf(a):
    import ml_dtypes
    return np.ascontiguousarray(np.asarray(a, dtype=np.float32)
                                .astype(ml_dtypes.bfloat16))


def make_in_maps(inputs):
    f = lambda a: np.ascontiguousarray(np.asarray(a), dtype=np.float32)
    seq = np.asarray(inputs["sequence"]).astype(np.int16)       # [B, S]
    emb = f(inputs["emb"])
    pesT = _bf(np.asarray(inputs["pes"], dtype=np.float32).T)   # [D, S] bf16
    Wq, Wk, Wv = (np.asarray(inputs[k]) for k in ("Wq", "Wk", "Wv"))
    Wo, W1, W2 = (np.asarray(inputs[k]) for k in ("Wo", "W1", "W2"))
    bq, bk, bv = f(inputs["bq"]), f(inputs["bk"]), f(inputs["bv"])
    b1 = f(inputs["b1"])
    g1, be1 = f(inputs["g1"]), f(inputs["be1"])
    g2, be2 = f(inputs["g2"]), f(inputs["be2"])

    in_maps = []
    for c in range(NC):
        b, t = c // TP, c % TP
        ds_ = slice(t * DSH, (t + 1) * DSH)
        fs_ = slice(t * FSH, (t + 1) * FSH)
        idx = np.ascontiguousarray(seq[b].reshape(SL // 16, 16).T)  # [16, 32]
        in_maps.append({
            "emb": emb,
            "idx": idx,
            "pesT": pesT,
            "wq": _bf(Wq[:, :, ds_]),
            "wk": _bf(Wk[:, :, ds_]),
            "wv": _bf(Wv[:, :, ds_]),
            "wo": _bf(Wo[:, ds_, :]),
            "w1": _bf(W1[:, :, fs_]),
            "w2": _bf(W2[:, fs_, :]),
            "bq": np.ascontiguousarray(bq[:, ds_]),
            "bk": np.ascontiguousarray(bk[:, ds_]),
            "bv": np.ascontiguousarray(bv[:, ds_]),
            "b1": np.ascontiguousarray(b1[:, fs_]),
            "g1": g1, "be1": be1, "g2": g2, "be2": be2,
        })
    return in_maps


def assemble(results):
    """[B,S,D] fp32 from per-core [D,SL] bf16 outs (cores 0,2,4,6)."""
    outs = []
    for b in range(B):
        o = np.asarray(results[TP * b]["out"]).astype(np.float32)  # [D, SL]
        outs.append(np.ascontiguousarray(o.T))                     # [SL, D]
    return np.stack(outs, axis=0)


_CACHE = {}


def _get_module():
    if "nc" not in _CACHE:
        _CACHE["nc"] = build_module()
    return _CACHE["nc"]


def kernel(**inputs):
    from concourse import bass_utils
    nc = _get_module()
    in_maps = make_in_maps(inputs)
    res = bass_utils.run_bass_kernel_spmd(nc, in_maps, list(range(NC)))
    return assemble(res.results)
